# revision 1
# baseline (speedup 1.0000x reference)
"""Multi-head attention (N=2, K=2048, M=1024, H=16, D=64) on 8 TRN2 cores.

Sharding: tensor-parallel over heads — core c owns heads (2c, 2c+1).
Each core computes q/k/v projections for its 2 heads (full sequence),
attention, and a rank-128 partial of the output projection (its 128 rows
of Wo's input dim). Host sums the 8 partials and adds bo. No device
collectives.

On-device layouts (per core):
  xq/xk/xv  [1024 m, 4096 tok] bf16   host-transposed inputs (tok = n*2048+k)
  wq/wk/wv  [1024 m, 128 hd]   bf16   W[h,d,m] -> [m, hl*64+d] for local heads
  wo        [128 hd, 1024 mo]  bf16   Wo[:, c*128:(c+1)*128].T
  bqk       [128, 2] f32, bv [128, 128] f32 (row-broadcast)
  outT      [1024 mo, 4096 tok] f32   partial output, transposed

Compute: qT/kT [hd,tok] and v [tok,hd] via PE (bf16, fp32 accum);
flash-style attention with transposed scores S^T[l,kq] so softmax-sum
comes free via a ones-row appended to v; exp on ScalarE straight out of
PSUM; normalization via DVE reciprocal + gpsimd partition_broadcast.
"""
import numpy as np
import ml_dtypes

import concourse.bass as bass
import concourse.tile as tile
from concourse import bacc, mybir
from concourse.bass_utils import run_bass_kernel_spmd

F32 = mybir.dt.float32
BF16 = mybir.dt.bfloat16
BFNP = ml_dtypes.bfloat16

N_CORES = 8
DM = 1024          # d_model
TOK = 4096         # N*K tokens
SEQ = 2048         # tokens per batch
NB = 2             # batches
HC = 2             # heads per core
D = 64             # head dim
EXPW = 1024        # exp tile width (kq per scores psum tile)

_prog_cache = {}


def build_program():
    nc = bacc.Bacc("TRN2", target_bir_lowering=False, debug=False)

    xq = nc.dram_tensor("xq", [DM, TOK], BF16, kind="ExternalInput")
    xk = nc.dram_tensor("xk", [DM, TOK], BF16, kind="ExternalInput")
    xv = nc.dram_tensor("xv", [DM, TOK], BF16, kind="ExternalInput")
    wq = nc.dram_tensor("wq", [DM, 128], BF16, kind="ExternalInput")
    wk = nc.dram_tensor("wk", [DM, 128], BF16, kind="ExternalInput")
    wv = nc.dram_tensor("wv", [DM, 128], BF16, kind="ExternalInput")
    wo = nc.dram_tensor("wo", [128, DM], BF16, kind="ExternalInput")
    bqk = nc.dram_tensor("bqk", [128, 2], F32, kind="ExternalInput")
    bv = nc.dram_tensor("bv", [128, 128], F32, kind="ExternalInput")
    outT = nc.dram_tensor("outT", [DM, TOK], F32, kind="ExternalOutput")

    Exp = mybir.ActivationFunctionType.Exp

    with tile.TileContext(nc) as tc:
        with (
            tc.tile_pool(name="const", bufs=1) as const,
            tc.tile_pool(name="big", bufs=1) as big,
            tc.tile_pool(name="xpool", bufs=24) as xpool,
            tc.tile_pool(name="xvpool", bufs=24) as xvpool,
            tc.tile_pool(name="attn", bufs=3) as attnp,
            tc.tile_pool(name="norm", bufs=2) as normp,
            tc.tile_pool(name="osb", bufs=4) as osb,
            tc.tile_pool(name="mm_ps", bufs=2, space="PSUM") as mm_ps,
            tc.tile_pool(name="sc_ps", bufs=1, space="PSUM") as sc_ps,
            tc.tile_pool(name="y_ps", bufs=2, space="PSUM") as y_ps,
        ):
            # ---- weights / biases ----
            wq_sb = const.tile([128, 8, 128], BF16, tag="wq")
            nc.sync.dma_start(wq_sb[:], wq.ap().rearrange("(c p) d -> p c d", p=128))
            wk_sb = const.tile([128, 8, 128], BF16, tag="wk")
            nc.sync.dma_start(wk_sb[:], wk.ap().rearrange("(c p) d -> p c d", p=128))
            wv_sb = const.tile([128, 8, 128], BF16, tag="wv")
            nc.sync.dma_start(wv_sb[:], wv.ap().rearrange("(c p) d -> p c d", p=128))
            wo_sb = const.tile([128, DM], BF16, tag="wo")
            nc.sync.dma_start(wo_sb[:], wo[:, :])
            bqk_sb = const.tile([128, 2], F32, tag="bqk")
            nc.sync.dma_start(bqk_sb[:], bqk[:, :])
            bv_sb = const.tile([128, 128], F32, tag="bv")
            nc.sync.dma_start(bv_sb[:], bv[:, :])

            # ---- persistent activations ----
            qT = big.tile([128, TOK], BF16, tag="qT")     # [hd, tok]
            kT = big.tile([128, TOK], BF16, tag="kT")     # [hd, tok]
            # v blocks: 32 token-blocks of [128 tok, 2*(64+1)]; col 64 of each
            # per-head group is the ones column (softmax denominator trick)
            vA = big.tile([128, 32 * 130], BF16, tag="vA")
            yT = big.tile([128, TOK], BF16, tag="yT")     # attn out [hd, tok]

            nc.vector.memset(
                vA[:].rearrange("p (b h c) -> p b h c", h=2, c=65)[:, :, :, 64:65], 1.0
            )

            def proj_qk(n):
                for tb in range(n * 4, n * 4 + 4):          # 512-token blocks
                    for xdram, w_sb, dstT, bcol in (
                        (xq, wq_sb, qT, 0),
                        (xk, wk_sb, kT, 1),
                    ):
                        ps = mm_ps.tile([128, 512], F32, tag="mm")
                        for mc in range(8):
                            xt = xpool.tile([128, 512], BF16, tag="xt")
                            nc.sync.dma_start(
                                xt[:],
                                xdram[mc * 128:(mc + 1) * 128,
                                      tb * 512:(tb + 1) * 512],
                            )
                            nc.tensor.matmul(
                                ps[:], lhsT=w_sb[:, mc, :], rhs=xt[:],
                                start=(mc == 0), stop=(mc == 7),
                            )
                        nc.vector.tensor_scalar_add(
                            dstT[:, tb * 512:(tb + 1) * 512], ps[:],
                            bqk_sb[:, bcol:bcol + 1],
                        )

            def proj_v(n):
                for tb2 in range(n * 16, n * 16 + 16):      # 128-token blocks
                    psv = mm_ps.tile([128, 128], F32, tag="mm")
                    for mc in range(8):
                        xvt = xvpool.tile([128, 128], BF16, tag="xvt")
                        nc.sync.dma_start(
                            xvt[:],
                            xv[mc * 128:(mc + 1) * 128,
                               tb2 * 128:(tb2 + 1) * 128],
                        )
                        nc.tensor.matmul(
                            psv[:], lhsT=xvt[:], rhs=wv_sb[:, mc, :],
                            start=(mc == 0), stop=(mc == 7),
                        )
                    base = tb2 * 130
                    for hl in range(2):
                        nc.vector.tensor_add(
                            vA[:, base + hl * 65: base + hl * 65 + 64],
                            psv[:, hl * 64:(hl + 1) * 64],
                            bv_sb[:, hl * 64:(hl + 1) * 64],
                        )

            def out_proj(n, tb_loc):
                tb = n * 4 + tb_loc
                for mb in range(8):
                    ps = mm_ps.tile([128, 512], F32, tag="mm")
                    nc.tensor.matmul(
                        ps[:], lhsT=wo_sb[:, mb * 128:(mb + 1) * 128],
                        rhs=yT[:, tb * 512:(tb + 1) * 512],
                        start=True, stop=True,
                    )
                    o_sb = osb.tile([128, 512], F32, tag="o")
                    nc.any.tensor_copy(o_sb[:], ps[:])
                    nc.sync.dma_start(
                        outT[mb * 128:(mb + 1) * 128, tb * 512:(tb + 1) * 512],
                        o_sb[:],
                    )

            def attention(n, h):
                hp = h * 64            # head partition offset in qT/kT
                for hf in range(2):    # kq half (1024 wide)
                    kq0 = n * SEQ + hf * EXPW
                    yacc = y_ps.tile([65, EXPW], F32, tag="yacc")
                    for lb in range(16):          # l chunks of 128 in batch n
                        lt = n * 16 + lb          # global 128-token block
                        sp = sc_ps.tile([128, EXPW], F32, tag="sc")
                        for qb in range(2):
                            nc.tensor.matmul(
                                sp[:, qb * 512:(qb + 1) * 512],
                                lhsT=kT[hp:hp + 64, lt * 128:(lt + 1) * 128],
                                rhs=qT[hp:hp + 64, kq0 + qb * 512: kq0 + (qb + 1) * 512],
                                start=True, stop=True,
                            )
                        at = attnp.tile([128, EXPW], BF16, tag="at")
                        nc.scalar.activation(at[:], sp[:], Exp, scale=0.125)
                        for qb in range(2):
                            nc.tensor.matmul(
                                yacc[:, qb * 512:(qb + 1) * 512],
                                lhsT=vA[:, lt * 130 + h * 65: lt * 130 + h * 65 + 65],
                                rhs=at[:, qb * 512:(qb + 1) * 512],
                                start=(lb == 0), stop=(lb == 15),
                            )
                    # normalize: yT[head, cols] = yacc[0:64] / yacc[64]
                    y_sb = normp.tile([65, EXPW], F32, tag="ysb")
                    nc.vector.tensor_copy(y_sb[:], yacc[:])
                    recip = normp.tile([1, EXPW], F32, tag="recip")
                    nc.vector.reciprocal(recip[:], y_sb[64:65, :])
                    bcast = normp.tile([64, EXPW], F32, tag="bcast")
                    nc.gpsimd.partition_broadcast(bcast[:], recip[:])
                    nc.vector.tensor_mul(
                        yT[hp:hp + 64, kq0:kq0 + EXPW], y_sb[0:64, :], bcast[:]
                    )
                    if h == 1:
                        # this 1024-token stripe of yT is complete for both
                        # heads -> emit its slice of the output projection
                        out_proj(n, hf * 2)
                        out_proj(n, hf * 2 + 1)

            for n in range(NB):
                proj_qk(n)
                proj_v(n)
                attention(n, 0)
                attention(n, 1)

    nc.compile()
    return nc


def get_program():
    if "nc" not in _prog_cache:
        _prog_cache["nc"] = build_program()
    return _prog_cache["nc"]


def make_in_maps(query, key, value, Wq, bq, Wk, bk, Wv, bv, Wo):
    """Host-side shard + layout. Returns list of 8 per-core input dicts."""
    xq = np.ascontiguousarray(query.reshape(TOK, DM).T).astype(BFNP)
    xk = np.ascontiguousarray(key.reshape(TOK, DM).T).astype(BFNP)
    xv = np.ascontiguousarray(value.reshape(TOK, DM).T).astype(BFNP)

    in_maps = []
    for c in range(N_CORES):
        h0 = HC * c
        # W[h,d,m] slice -> [m, hl*64+d]
        wq_c = np.ascontiguousarray(
            np.transpose(Wq[h0:h0 + HC], (2, 0, 1)).reshape(DM, 128)).astype(BFNP)
        wk_c = np.ascontiguousarray(
            np.transpose(Wk[h0:h0 + HC], (2, 0, 1)).reshape(DM, 128)).astype(BFNP)
        wv_c = np.ascontiguousarray(
            np.transpose(Wv[h0:h0 + HC], (2, 0, 1)).reshape(DM, 128)).astype(BFNP)
        wo_c = np.ascontiguousarray(
            Wo[:, 128 * c:128 * (c + 1)].T).astype(BFNP)
        bqk_c = np.stack(
            [bq[h0:h0 + HC].reshape(128), bk[h0:h0 + HC].reshape(128)], axis=1
        ).astype(np.float32)
        bv_c = np.ascontiguousarray(
            np.broadcast_to(bv[h0:h0 + HC].reshape(1, 128), (128, 128))
        ).astype(np.float32)
        in_maps.append({
            "xq": xq, "xk": xk, "xv": xv,
            "wq": wq_c, "wk": wk_c, "wv": wv_c, "wo": wo_c,
            "bqk": bqk_c, "bv": bv_c,
        })
    return in_maps


def kernel(query, key, value, Wq, bq, Wk, bk, Wv, bv, Wo, bo):
    nc = get_program()
    in_maps = make_in_maps(query, key, value, Wq, bq, Wk, bk, Wv, bv, Wo)
    res = run_bass_kernel_spmd(nc, in_maps, list(range(N_CORES)))
    acc = np.zeros((DM, TOK), np.float32)
    for c in range(N_CORES):
        acc += res.results[c]["outT"]
    out = acc.T.reshape(NB, SEQ, DM) + bo.astype(np.float32)
    return out


# revision 3
# speedup vs baseline: 1.6545x; 1.6545x over previous
"""Multi-head attention (N=2, K=2048, M=1024, H=16, D=64) on 8 TRN2 cores.

Sharding: tensor-parallel over heads — core c owns heads (2c, 2c+1).
Each core computes q/k/v projections for its 2 heads (full sequence),
attention, and a rank-128 partial of the output projection (its 128 rows
of Wo's input dim). Host sums the 8 partials and adds bo. No device
collectives.

On-device layouts (per core):
  xq/xk/xv [8 mc, 8 tb, 128, 512] bf16  host-tiled transposed inputs:
           tile (mc, tb)[p, f] = x[tok=tb*512+f, m=mc*128+p], tok = n*2048+k
  wq/wk/wv [1024 m, 128 hd] bf16   W[h,d,m] -> [m, hl*64+d] for local heads
  wo       [128 hd, 1024 mo] bf16  Wo[:, c*128:(c+1)*128].T
  bqk      [128, 2] f32, bv [128, 128] f32 (row-broadcast)
  outT     [8 mb, 8 tb, 128, 512] f32  tiled partial output

Compute: qT/kT [hd,tok] and v [tok,hd] via PE (bf16, fp32 accum);
flash-style attention with transposed scores S^T[l,kq] so the softmax
denominator comes free via a ones-row appended to v; exp on ScalarE
straight out of PSUM; normalization via DVE reciprocal_approx_fast +
gpsimd partition_broadcast.
"""
import numpy as np
import ml_dtypes

import concourse.bass as bass
import concourse.tile as tile
from concourse import bacc, mybir
from concourse.bass_utils import run_bass_kernel_spmd

F32 = mybir.dt.float32
BF16 = mybir.dt.bfloat16
BFNP = ml_dtypes.bfloat16

N_CORES = 8
DM = 1024          # d_model
TOK = 4096         # N*K tokens
SEQ = 2048         # tokens per batch
NB = 2             # batches
HC = 2             # heads per core
D = 64             # head dim
EXPW = 1024        # exp tile width (kq per scores psum tile)

_prog_cache = {}


def build_program():
    nc = bacc.Bacc("TRN2", target_bir_lowering=False, debug=False)

    xq = nc.dram_tensor("xq", [8, 8, 128, 512], BF16, kind="ExternalInput")
    xk = nc.dram_tensor("xk", [8, 8, 128, 512], BF16, kind="ExternalInput")
    xv = nc.dram_tensor("xv", [8, 8, 128, 512], BF16, kind="ExternalInput")
    wq = nc.dram_tensor("wq", [DM, 128], BF16, kind="ExternalInput")
    wk = nc.dram_tensor("wk", [DM, 128], BF16, kind="ExternalInput")
    wv = nc.dram_tensor("wv", [DM, 128], BF16, kind="ExternalInput")
    wo = nc.dram_tensor("wo", [128, DM], BF16, kind="ExternalInput")
    bqk = nc.dram_tensor("bqk", [128, 2], F32, kind="ExternalInput")
    bv = nc.dram_tensor("bv", [128, 128], F32, kind="ExternalInput")
    outT = nc.dram_tensor("outT", [8, 8, 128, 512], F32, kind="ExternalOutput")

    Exp = mybir.ActivationFunctionType.Exp

    with tile.TileContext(nc) as tc:
        with (
            tc.tile_pool(name="const", bufs=1) as const,
            tc.tile_pool(name="big", bufs=1) as big,
            tc.tile_pool(name="xpool", bufs=24) as xpool,
            tc.tile_pool(name="xvpool", bufs=12) as xvpool,
            tc.tile_pool(name="attn", bufs=3) as attnp,
            tc.tile_pool(name="norm", bufs=2) as normp,
            tc.tile_pool(name="osb", bufs=4) as osb,
            tc.tile_pool(name="mm_ps", bufs=2, space="PSUM") as mm_ps,
            tc.tile_pool(name="sc_ps", bufs=2, space="PSUM") as sc_ps,
            tc.tile_pool(name="y_ps", bufs=1, space="PSUM") as y_ps,
        ):
            # ---- weights / biases ----
            wq_sb = const.tile([128, 8, 128], BF16, tag="wq")
            nc.sync.dma_start(wq_sb[:], wq.ap().rearrange("(c p) d -> p c d", p=128))
            wk_sb = const.tile([128, 8, 128], BF16, tag="wk")
            nc.sync.dma_start(wk_sb[:], wk.ap().rearrange("(c p) d -> p c d", p=128))
            wv_sb = const.tile([128, 8, 128], BF16, tag="wv")
            nc.sync.dma_start(wv_sb[:], wv.ap().rearrange("(c p) d -> p c d", p=128))
            wo_sb = const.tile([128, DM], BF16, tag="wo")
            nc.sync.dma_start(wo_sb[:], wo[:, :])
            bqk_sb = const.tile([128, 2], F32, tag="bqk")
            nc.sync.dma_start(bqk_sb[:], bqk[:, :])
            bv_sb = const.tile([128, 128], F32, tag="bv")
            nc.sync.dma_start(bv_sb[:], bv[:, :])

            # ---- persistent activations ----
            qT = big.tile([128, TOK], BF16, tag="qT")     # [hd, tok]
            kT = big.tile([128, TOK], BF16, tag="kT")     # [hd, tok]
            # v blocks: 32 token-blocks of [128 tok, 2*(64+1)]; col 64 of each
            # per-head group is the ones column (softmax denominator trick)
            vA = big.tile([128, 32 * 130], BF16, tag="vA")
            yT = big.tile([128, TOK], BF16, tag="yT")     # attn out [hd, tok]

            nc.vector.memset(
                vA[:].rearrange("p (b h c) -> p b h c", h=2, c=65)[:, :, :, 64:65], 1.0
            )

            def proj_qk(n):
                for tb in range(n * 4, n * 4 + 4):          # 512-token blocks
                    for xdram, w_sb, dstT, bcol in (
                        (xq, wq_sb, qT, 0),
                        (xk, wk_sb, kT, 1),
                    ):
                        ps = mm_ps.tile([128, 512], F32, tag="mm")
                        for mc in range(8):
                            xt = xpool.tile([128, 512], BF16, tag="xt")
                            nc.sync.dma_start(xt[:], xdram[mc, tb])
                            nc.tensor.matmul(
                                ps[:], lhsT=w_sb[:, mc, :], rhs=xt[:],
                                start=(mc == 0), stop=(mc == 7),
                            )
                        nc.vector.tensor_scalar_add(
                            dstT[:, tb * 512:(tb + 1) * 512], ps[:],
                            bqk_sb[:, bcol:bcol + 1],
                        )

            def proj_v(n):
                for tb in range(n * 4, n * 4 + 4):          # 512-token blocks
                    xvt = []
                    for mc in range(8):
                        t = xvpool.tile([128, 512], BF16, tag="xvt")
                        nc.sync.dma_start(t[:], xv[mc, tb])
                        xvt.append(t)
                    for j in range(4):                      # 128-token blocks
                        tb2 = tb * 4 + j
                        psv = mm_ps.tile([128, 128], F32, tag="mm")
                        for mc in range(8):
                            nc.tensor.matmul(
                                psv[:], lhsT=xvt[mc][:, j * 128:(j + 1) * 128],
                                rhs=wv_sb[:, mc, :],
                                start=(mc == 0), stop=(mc == 7),
                            )
                        base = tb2 * 130
                        for hl in range(2):
                            nc.vector.tensor_add(
                                vA[:, base + hl * 65: base + hl * 65 + 64],
                                psv[:, hl * 64:(hl + 1) * 64],
                                bv_sb[:, hl * 64:(hl + 1) * 64],
                            )

            def out_proj(n, tb_loc):
                tb = n * 4 + tb_loc
                for mb in range(8):
                    ps = mm_ps.tile([128, 512], F32, tag="mm")
                    nc.tensor.matmul(
                        ps[:], lhsT=wo_sb[:, mb * 128:(mb + 1) * 128],
                        rhs=yT[:, tb * 512:(tb + 1) * 512],
                        start=True, stop=True,
                    )
                    o_sb = osb.tile([128, 512], F32, tag="o")
                    nc.any.tensor_copy(o_sb[:], ps[:])
                    nc.sync.dma_start(outT[mb, tb], o_sb[:])

            def attention(n, h):
                hp = h * 64            # head partition offset in qT/kT
                for hf in range(2):    # kq half (1024 wide)
                    kq0 = n * SEQ + hf * EXPW
                    yacc = y_ps.tile([65, EXPW], F32, tag="yacc")
                    for lb in range(16):          # l chunks of 128 in batch n
                        lt = n * 16 + lb          # global 128-token block
                        sp = sc_ps.tile([128, EXPW], F32, tag="sc")
                        for qb in range(2):
                            nc.tensor.matmul(
                                sp[:, qb * 512:(qb + 1) * 512],
                                lhsT=kT[hp:hp + 64, lt * 128:(lt + 1) * 128],
                                rhs=qT[hp:hp + 64, kq0 + qb * 512: kq0 + (qb + 1) * 512],
                                start=True, stop=True,
                            )
                        at = attnp.tile([128, EXPW], BF16, tag="at")
                        nc.scalar.activation(at[:], sp[:], Exp, scale=0.125)
                        for qb in range(2):
                            nc.tensor.matmul(
                                yacc[:, qb * 512:(qb + 1) * 512],
                                lhsT=vA[:, lt * 130 + h * 65: lt * 130 + h * 65 + 65],
                                rhs=at[:, qb * 512:(qb + 1) * 512],
                                start=(lb == 0), stop=(lb == 15),
                            )
                    # normalize: yT[head, cols] = yacc[0:64] / yacc[64]
                    y_sb = normp.tile([65, EXPW], F32, tag="ysb")
                    nc.vector.tensor_copy(y_sb[:], yacc[:])
                    recip = normp.tile([1, EXPW], F32, tag="recip")
                    nc.vector.reciprocal(recip[:], y_sb[64:65, :])
                    bcast = normp.tile([64, EXPW], F32, tag="bcast")
                    nc.gpsimd.partition_broadcast(bcast[:], recip[:])
                    nc.vector.tensor_mul(
                        yT[hp:hp + 64, kq0:kq0 + EXPW], y_sb[0:64, :], bcast[:]
                    )
                    if h == 1:
                        # this 1024-token stripe of yT is complete for both
                        # heads -> emit its slice of the output projection
                        out_proj(n, hf * 2)
                        out_proj(n, hf * 2 + 1)

            for n in range(NB):
                proj_qk(n)
                proj_v(n)
                attention(n, 0)
                attention(n, 1)

    nc.compile()
    return nc


def get_program():
    if "nc" not in _prog_cache:
        _prog_cache["nc"] = build_program()
    return _prog_cache["nc"]


def _tile_x(x):
    # [TOK, DM] f32 -> bf16 tiles [8 mc, 8 tb, 128, 512]: t[mc,tb,p,f] =
    # x[tb*512+f, mc*128+p]
    t = x.reshape(8, 512, 8, 128).astype(BFNP)   # [tb, f, mc, p]
    return np.ascontiguousarray(np.transpose(t, (2, 0, 3, 1)))


def make_in_maps(query, key, value, Wq, bq, Wk, bk, Wv, bv, Wo):
    """Host-side shard + layout. Returns list of 8 per-core input dicts."""
    xq = _tile_x(query.reshape(TOK, DM))
    xk = _tile_x(key.reshape(TOK, DM))
    xv = _tile_x(value.reshape(TOK, DM))

    in_maps = []
    for c in range(N_CORES):
        h0 = HC * c
        # W[h,d,m] slice -> [m, hl*64+d]
        wq_c = np.ascontiguousarray(
            np.transpose(Wq[h0:h0 + HC], (2, 0, 1)).reshape(DM, 128)).astype(BFNP)
        wk_c = np.ascontiguousarray(
            np.transpose(Wk[h0:h0 + HC], (2, 0, 1)).reshape(DM, 128)).astype(BFNP)
        wv_c = np.ascontiguousarray(
            np.transpose(Wv[h0:h0 + HC], (2, 0, 1)).reshape(DM, 128)).astype(BFNP)
        wo_c = np.ascontiguousarray(
            Wo[:, 128 * c:128 * (c + 1)].T).astype(BFNP)
        bqk_c = np.stack(
            [bq[h0:h0 + HC].reshape(128), bk[h0:h0 + HC].reshape(128)], axis=1
        ).astype(np.float32)
        bv_c = np.ascontiguousarray(
            np.broadcast_to(bv[h0:h0 + HC].reshape(1, 128), (128, 128))
        ).astype(np.float32)
        in_maps.append({
            "xq": xq, "xk": xk, "xv": xv,
            "wq": wq_c, "wk": wk_c, "wv": wv_c, "wo": wo_c,
            "bqk": bqk_c, "bv": bv_c,
        })
    return in_maps


def untile_out(res_list):
    """Sum per-core tiled partials -> [DM, TOK] f32."""
    acc = np.zeros((8, 8, 128, 512), np.float32)
    for r in res_list:
        acc += r["outT"]
    # [mb, tb, p, f] -> [mb*128+p, tb*512+f]
    return np.ascontiguousarray(np.transpose(acc, (0, 2, 1, 3))).reshape(DM, TOK)


def kernel(query, key, value, Wq, bq, Wk, bk, Wv, bv, Wo, bo):
    nc = get_program()
    in_maps = make_in_maps(query, key, value, Wq, bq, Wk, bk, Wv, bv, Wo)
    res = run_bass_kernel_spmd(nc, in_maps, list(range(N_CORES)))
    out_t = untile_out(res.results)
    out = out_t.T.reshape(NB, SEQ, DM) + bo.astype(np.float32)
    return out


# revision 4
# speedup vs baseline: 1.8777x; 1.1349x over previous
"""Multi-head attention (N=2, K=2048, M=1024, H=16, D=64) on 8 TRN2 cores.

Sharding: tensor-parallel over heads — core c owns heads (2c, 2c+1).
Each core computes q/k/v projections for its 2 heads (full sequence),
attention, and a rank-128 partial of the output projection (its 128 rows
of Wo's input dim). Host sums the 8 partials and adds bo. No device
collectives.

On-device layouts (per core):
  xq/xk/xv [8 tb, 128 p, 8 mc, 512 f] bf16  host-tiled transposed inputs:
           [tb, p, mc, f] = x[tok=tb*512+f, m=mc*128+p], tok = n*2048+k
           -> one contiguous 1MB DMA per (tensor, tb)
  wq/wk/wv [1024 m, 128 hd] bf16   W[h,d,m] -> [m, hl*64+d] for local heads
  wo       [128 hd, 1024 mo] bf16  Wo[:, c*128:(c+1)*128].T
  bqk      [128, 2] f32, bv [128, 128] f32 (row-broadcast)
  outT     [8 tb, 128 p, 8 mb, 512 f] f32  tiled partial (2MB DMA per tb)

Compute: qT/kT [hd,tok] and v [tok,hd] via PE (bf16, fp32 accum);
flash-style attention with transposed scores S^T[l,kq] so the softmax
denominator comes free via a ones-row appended to v; exp on ScalarE
straight out of PSUM; normalization via DVE reciprocal + gpsimd
partition_broadcast. Batch-1 projections are emitted interleaved into
batch-0 attention so PE fills ScalarE-bound gaps and ACT never idles
between batches.
"""
import numpy as np
import ml_dtypes

import concourse.bass as bass
import concourse.tile as tile
from concourse import bacc, mybir
from concourse.bass_utils import run_bass_kernel_spmd

F32 = mybir.dt.float32
BF16 = mybir.dt.bfloat16
BFNP = ml_dtypes.bfloat16

N_CORES = 8
DM = 1024          # d_model
TOK = 4096         # N*K tokens
SEQ = 2048         # tokens per batch
NB = 2             # batches
HC = 2             # heads per core
D = 64             # head dim
EXPW = 1024        # exp tile width (kq per scores psum tile)

_prog_cache = {}


def build_program():
    nc = bacc.Bacc("TRN2", target_bir_lowering=False, debug=False)

    xq = nc.dram_tensor("xq", [8, 128, 8, 512], BF16, kind="ExternalInput")
    xk = nc.dram_tensor("xk", [8, 128, 8, 512], BF16, kind="ExternalInput")
    xv = nc.dram_tensor("xv", [8, 128, 8, 512], BF16, kind="ExternalInput")
    wq = nc.dram_tensor("wq", [DM, 128], BF16, kind="ExternalInput")
    wk = nc.dram_tensor("wk", [DM, 128], BF16, kind="ExternalInput")
    wv = nc.dram_tensor("wv", [DM, 128], BF16, kind="ExternalInput")
    wo = nc.dram_tensor("wo", [128, DM], BF16, kind="ExternalInput")
    bqk = nc.dram_tensor("bqk", [128, 2], F32, kind="ExternalInput")
    bv = nc.dram_tensor("bv", [128, 128], F32, kind="ExternalInput")
    outT = nc.dram_tensor("outT", [8, 128, 8, 512], F32, kind="ExternalOutput")

    Exp = mybir.ActivationFunctionType.Exp

    with tile.TileContext(nc) as tc:
        with (
            tc.tile_pool(name="const", bufs=1) as const,
            tc.tile_pool(name="big", bufs=1) as big,
            tc.tile_pool(name="xpool", bufs=12) as xpool,
            tc.tile_pool(name="attn", bufs=3) as attnp,
            tc.tile_pool(name="norm", bufs=2) as normp,
            tc.tile_pool(name="osb", bufs=1) as osb,
            tc.tile_pool(name="mm_ps", bufs=2, space="PSUM") as mm_ps,
            tc.tile_pool(name="sc_ps", bufs=2, space="PSUM") as sc_ps,
            tc.tile_pool(name="y_ps", bufs=1, space="PSUM") as y_ps,
        ):
            # ---- weights / biases ----
            wq_sb = const.tile([128, 8, 128], BF16, tag="wq")
            nc.sync.dma_start(wq_sb[:], wq.ap().rearrange("(c p) d -> p c d", p=128))
            wk_sb = const.tile([128, 8, 128], BF16, tag="wk")
            nc.sync.dma_start(wk_sb[:], wk.ap().rearrange("(c p) d -> p c d", p=128))
            wv_sb = const.tile([128, 8, 128], BF16, tag="wv")
            nc.sync.dma_start(wv_sb[:], wv.ap().rearrange("(c p) d -> p c d", p=128))
            wo_sb = const.tile([128, DM], BF16, tag="wo")
            nc.sync.dma_start(wo_sb[:], wo[:, :])
            bqk_sb = const.tile([128, 2], F32, tag="bqk")
            nc.sync.dma_start(bqk_sb[:], bqk[:, :])
            bv_sb = const.tile([128, 128], F32, tag="bv")
            nc.sync.dma_start(bv_sb[:], bv[:, :])

            # ---- persistent activations ----
            qT = big.tile([128, TOK], BF16, tag="qT")     # [hd, tok]
            kT = big.tile([128, TOK], BF16, tag="kT")     # [hd, tok]
            # v blocks: 32 token-blocks of [128 tok, 2*(64+1)]; col 64 of each
            # per-head group is the ones column (softmax denominator trick)
            vA = big.tile([128, 32 * 130], BF16, tag="vA")
            yT = big.tile([128, TOK], BF16, tag="yT")     # attn out [hd, tok]

            nc.vector.memset(
                vA[:].rearrange("p (b h c) -> p b h c", h=2, c=65)[:, :, :, 64:65], 1.0
            )

            def proj_qk(tb):
                for xdram, w_sb, dstT, bcol in (
                    (xq, wq_sb, qT, 0),
                    (xk, wk_sb, kT, 1),
                ):
                    xt = xpool.tile([128, 8, 512], BF16, tag="xt")
                    nc.sync.dma_start(xt[:], xdram[tb])
                    ps = mm_ps.tile([128, 512], F32, tag="mm")
                    for mc in range(8):
                        nc.tensor.matmul(
                            ps[:], lhsT=w_sb[:, mc, :], rhs=xt[:, mc, :],
                            start=(mc == 0), stop=(mc == 7),
                        )
                    nc.vector.tensor_scalar_add(
                        dstT[:, tb * 512:(tb + 1) * 512], ps[:],
                        bqk_sb[:, bcol:bcol + 1],
                    )

            def proj_v(tb):
                xt = xpool.tile([128, 8, 512], BF16, tag="xt")
                nc.sync.dma_start(xt[:], xv[tb])
                for j in range(4):                      # 128-token blocks
                    tb2 = tb * 4 + j
                    psv = mm_ps.tile([128, 128], F32, tag="mm")
                    for mc in range(8):
                        nc.tensor.matmul(
                            psv[:], lhsT=xt[:, mc, j * 128:(j + 1) * 128],
                            rhs=wv_sb[:, mc, :],
                            start=(mc == 0), stop=(mc == 7),
                        )
                    base = tb2 * 130
                    for hl in range(2):
                        nc.vector.tensor_add(
                            vA[:, base + hl * 65: base + hl * 65 + 64],
                            psv[:, hl * 64:(hl + 1) * 64],
                            bv_sb[:, hl * 64:(hl + 1) * 64],
                        )

            def proj_chunk(n, k):
                # quarter k of batch n's projections, for interleaved emission
                if k == 0:
                    proj_qk(n * 4 + 0); proj_qk(n * 4 + 1)
                elif k == 1:
                    proj_qk(n * 4 + 2); proj_qk(n * 4 + 3)
                elif k == 2:
                    proj_v(n * 4 + 0); proj_v(n * 4 + 1)
                else:
                    proj_v(n * 4 + 2); proj_v(n * 4 + 3)

            def out_proj(n, tb_loc):
                tb = n * 4 + tb_loc
                o_sb = osb.tile([128, 8, 512], F32, tag="o")
                for mb in range(8):
                    ps = mm_ps.tile([128, 512], F32, tag="mm")
                    nc.tensor.matmul(
                        ps[:], lhsT=wo_sb[:, mb * 128:(mb + 1) * 128],
                        rhs=yT[:, tb * 512:(tb + 1) * 512],
                        start=True, stop=True,
                    )
                    nc.vector.tensor_copy(o_sb[:, mb, :], ps[:])
                nc.sync.dma_start(outT[tb], o_sb[:])

            def attention_combo(n, h, hf):
                hp = h * 64            # head partition offset in qT/kT
                kq0 = n * SEQ + hf * EXPW
                yacc = y_ps.tile([65, EXPW], F32, tag="yacc")
                for lb in range(16):          # l chunks of 128 in batch n
                    lt = n * 16 + lb          # global 128-token block
                    sp = sc_ps.tile([128, EXPW], F32, tag="sc")
                    for qb in range(2):
                        nc.tensor.matmul(
                            sp[:, qb * 512:(qb + 1) * 512],
                            lhsT=kT[hp:hp + 64, lt * 128:(lt + 1) * 128],
                            rhs=qT[hp:hp + 64, kq0 + qb * 512: kq0 + (qb + 1) * 512],
                            start=True, stop=True,
                        )
                    at = attnp.tile([128, EXPW], BF16, tag="at")
                    nc.scalar.activation(at[:], sp[:], Exp, scale=0.125)
                    for qb in range(2):
                        nc.tensor.matmul(
                            yacc[:, qb * 512:(qb + 1) * 512],
                            lhsT=vA[:, lt * 130 + h * 65: lt * 130 + h * 65 + 65],
                            rhs=at[:, qb * 512:(qb + 1) * 512],
                            start=(lb == 0), stop=(lb == 15),
                        )
                # normalize: yT[head, cols] = yacc[0:64] / yacc[64]
                y_sb = normp.tile([65, EXPW], F32, tag="ysb")
                nc.vector.tensor_copy(y_sb[:], yacc[:])
                recip = normp.tile([1, EXPW], F32, tag="recip")
                nc.vector.reciprocal(recip[:], y_sb[64:65, :])
                bcast = normp.tile([64, EXPW], F32, tag="bcast")
                nc.gpsimd.partition_broadcast(bcast[:], recip[:])
                nc.vector.tensor_mul(
                    yT[hp:hp + 64, kq0:kq0 + EXPW], y_sb[0:64, :], bcast[:]
                )
                if h == 1:
                    # this 1024-token stripe of yT is complete for both
                    # heads -> emit its slice of the output projection
                    out_proj(n, hf * 2)
                    out_proj(n, hf * 2 + 1)

            for k in range(4):
                proj_chunk(0, k)
            for n in range(NB):
                ck = 0
                for h in range(HC):
                    for hf in range(2):
                        attention_combo(n, h, hf)
                        if n + 1 < NB:
                            proj_chunk(n + 1, ck)
                            ck += 1

    nc.compile()
    return nc


def get_program():
    if "nc" not in _prog_cache:
        _prog_cache["nc"] = build_program()
    return _prog_cache["nc"]


def _tile_x(x):
    # [TOK, DM] f32 -> bf16 tiles [8 tb, 128 p, 8 mc, 512 f]:
    # t[tb,p,mc,f] = x[tb*512+f, mc*128+p]
    t = x.reshape(8, 512, 8, 128).astype(BFNP)   # [tb, f, mc, p]
    return np.ascontiguousarray(np.transpose(t, (0, 3, 2, 1)))


def make_in_maps(query, key, value, Wq, bq, Wk, bk, Wv, bv, Wo):
    """Host-side shard + layout. Returns list of 8 per-core input dicts."""
    xq = _tile_x(query.reshape(TOK, DM))
    xk = _tile_x(key.reshape(TOK, DM))
    xv = _tile_x(value.reshape(TOK, DM))

    in_maps = []
    for c in range(N_CORES):
        h0 = HC * c
        # W[h,d,m] slice -> [m, hl*64+d]
        wq_c = np.ascontiguousarray(
            np.transpose(Wq[h0:h0 + HC], (2, 0, 1)).reshape(DM, 128)).astype(BFNP)
        wk_c = np.ascontiguousarray(
            np.transpose(Wk[h0:h0 + HC], (2, 0, 1)).reshape(DM, 128)).astype(BFNP)
        wv_c = np.ascontiguousarray(
            np.transpose(Wv[h0:h0 + HC], (2, 0, 1)).reshape(DM, 128)).astype(BFNP)
        wo_c = np.ascontiguousarray(
            Wo[:, 128 * c:128 * (c + 1)].T).astype(BFNP)
        bqk_c = np.stack(
            [bq[h0:h0 + HC].reshape(128), bk[h0:h0 + HC].reshape(128)], axis=1
        ).astype(np.float32)
        bv_c = np.ascontiguousarray(
            np.broadcast_to(bv[h0:h0 + HC].reshape(1, 128), (128, 128))
        ).astype(np.float32)
        in_maps.append({
            "xq": xq, "xk": xk, "xv": xv,
            "wq": wq_c, "wk": wk_c, "wv": wv_c, "wo": wo_c,
            "bqk": bqk_c, "bv": bv_c,
        })
    return in_maps


def untile_out(res_list):
    """Sum per-core tiled partials -> [DM, TOK] f32."""
    acc = np.zeros((8, 128, 8, 512), np.float32)
    for r in res_list:
        acc += r["outT"]
    # [tb, p, mb, f] -> [mb*128+p, tb*512+f]
    return np.ascontiguousarray(np.transpose(acc, (2, 1, 0, 3))).reshape(DM, TOK)


def kernel(query, key, value, Wq, bq, Wk, bk, Wv, bv, Wo, bo):
    nc = get_program()
    in_maps = make_in_maps(query, key, value, Wq, bq, Wk, bk, Wv, bv, Wo)
    res = run_bass_kernel_spmd(nc, in_maps, list(range(N_CORES)))
    out_t = untile_out(res.results)
    out = out_t.T.reshape(NB, SEQ, DM) + bo.astype(np.float32)
    return out
